# revision 12
# baseline (speedup 1.0000x reference)
"""Cross-modal multi-head attention on 8 Trainium2 NeuronCores.

- data-parallel over batch: 1 element per core, no collectives
- host pre-casts q/k/v and weights to bf16 -> DMA volume halves and
  qT/kT/vT come straight from dma_start_transpose (no PE transposes)
- head-PAIR processing: even head in PE rows 0:64, odd head in rows 64:128,
  score matmuls interleaved so both row-groups run concurrently on HW
- ones column appended per head in V -> PV matmul row 64 = softmax sums
- normalize: DVE reciprocal of sums + GpSimd partition_broadcast + multiply
- block software pipeline: during block b's head loop we interleave the
  output projection of block b-1 and the q-transpose/Q-proj of b+1
"""

import numpy as np

HEADS = 16
DM = 1024
IMG = 512
DK = 64
LQ = 2048
LKV = 1024
B = 8
P = 128
IB = 512

_cache = {}
TRACE = False
LAST_RESULT = None


def _build_nc(lq=LQ, lkv=LKV):
    from contextlib import ExitStack

    import concourse.tile as tile
    from concourse import bacc, library_config, mybir

    dt = mybir.dt
    f32 = dt.float32
    bf16 = dt.bfloat16
    Exp = mybir.ActivationFunctionType.Exp
    Alu = mybir.AluOpType

    n_jt = lkv // P
    n_ib = lq // IB
    n_it = IB // P
    n_do = DM // P
    n_dk = IMG // P
    HW = DK + 1

    nc = bacc.Bacc("TRN2", target_bir_lowering=False, debug=False)

    q = nc.declare_dram_parameter("q", [lq, DM], bf16, isOutput=False)
    k = nc.declare_dram_parameter("k", [lkv, IMG], bf16, isOutput=False)
    v = nc.declare_dram_parameter("v", [lkv, IMG], bf16, isOutput=False)
    Wq = nc.declare_dram_parameter("Wq", [DM, DM], bf16, isOutput=False)
    bq = nc.declare_dram_parameter("bq", [DM], f32, isOutput=False)
    Wk = nc.declare_dram_parameter("Wk", [IMG, DM], bf16, isOutput=False)
    bk = nc.declare_dram_parameter("bk", [DM], f32, isOutput=False)
    Wv = nc.declare_dram_parameter("Wv", [IMG, DM], bf16, isOutput=False)
    bv = nc.declare_dram_parameter("bv", [DM], f32, isOutput=False)
    Wo = nc.declare_dram_parameter("Wo", [DM, DM], bf16, isOutput=False)
    bo = nc.declare_dram_parameter("bo", [DM], f32, isOutput=False)
    out = nc.declare_dram_parameter("out", [lq, DM], f32, isOutput=True)

    with tile.TileContext(nc) as tc, ExitStack() as ctx:
        singles = ctx.enter_context(tc.tile_pool(name="singles", bufs=1))
        qT_pool = ctx.enter_context(tc.tile_pool(name="qT", bufs=2))
        qhT_pool = ctx.enter_context(tc.tile_pool(name="qhT", bufs=2))

        # PSUM: 2+2+1+1+2 = 8 banks
        psA = ctx.enter_context(tc.tile_pool(name="psA", bufs=1, space="PSUM"))
        psB = ctx.enter_context(tc.tile_pool(name="psB", bufs=1, space="PSUM"))
        psOA = ctx.enter_context(tc.tile_pool(name="psOA", bufs=1, space="PSUM"))
        psOB = ctx.enter_context(tc.tile_pool(name="psOB", bufs=1, space="PSUM"))
        ps_g = ctx.enter_context(tc.tile_pool(name="ps_g", bufs=2, space="PSUM"))

        Wq_sb = singles.tile([P, n_do, DM], bf16)
        Wk_sb = singles.tile([P, n_dk, DM], bf16)
        Wv_sb = singles.tile([P, n_dk, DM], bf16)
        Wo_sb = singles.tile([P, n_do, DM], bf16)
        bq_sb = singles.tile([P, n_do], f32)
        bk_sb = singles.tile([P, n_do], f32)
        bv_rep = singles.tile([P, DM], f32)
        bo_rep = singles.tile([P, DM], f32)
        khT_sb = singles.tile([P, n_do, lkv], bf16)
        vh_sb = singles.tile([P, n_jt, HEADS * HW], bf16)
        kT_sb = singles.tile([P, n_dk, lkv], bf16, tag="kT")
        vT_sb = singles.tile([P, n_dk, lkv], bf16, tag="vT")

        nc.gpsimd.load_library(library_config.proxy)

        def kp_chunk(do):
            def run():
                for jb in range(lkv // 512):
                    pp = ps_g.tile([P, 512], f32, tag="psg", name=f"kp_{do}_{jb}")
                    for dki in range(n_dk):
                        nc.tensor.matmul(
                            pp,
                            lhsT=Wk_sb[:, dki, do * P : (do + 1) * P],
                            rhs=kT_sb[:, dki, jb * 512 : (jb + 1) * 512],
                            start=(dki == 0),
                            stop=(dki == n_dk - 1),
                        )
                    nc.vector.tensor_scalar_add(
                        khT_sb[:, do, jb * 512 : (jb + 1) * 512],
                        pp,
                        bk_sb[:, do : do + 1],
                    )
            return run

        def vproj_chunk(jt, db):
            def run():
                pp = ps_g.tile([P, 512], f32, tag="psg", name=f"vp_{jt}_{db}")
                for dki in range(n_dk):
                    nc.tensor.matmul(
                        pp,
                        lhsT=vT_sb[:, dki, jt * P : (jt + 1) * P],
                        rhs=Wv_sb[:, dki, db * 512 : (db + 1) * 512],
                        start=(dki == 0),
                        stop=(dki == n_dk - 1),
                    )
                dst = vh_sb[:, jt, :].rearrange("p (h c) -> p h c", c=HW)[
                    :, db * 8 : (db + 1) * 8, :DK
                ]
                nc.vector.tensor_tensor(
                    dst,
                    pp.rearrange("p (h d) -> p h d", d=DK),
                    bv_rep[:, db * 512 : (db + 1) * 512].rearrange(
                        "p (h d) -> p h d", d=DK
                    ),
                    Alu.add,
                )
            return run

        def make_prep(ib):
            i0 = ib * IB
            qT_sb = qT_pool.tile([P, n_do, IB], bf16, tag="qT", name=f"qT_{ib}")
            qhT_sb = qhT_pool.tile([P, n_do, IB], bf16, tag="qhT", name=f"qhT_{ib}")
            chunks = []

            def tr_chunk():
                nc.sync.dma_start_transpose(qT_sb, q[i0 : i0 + IB, :])

            def proj_chunk(do):
                def run():
                    pp = ps_g.tile([P, 512], f32, tag="psg", name=f"qp_{ib}_{do}")
                    for dki in range(n_do):
                        nc.tensor.matmul(
                            pp,
                            lhsT=Wq_sb[:, dki, do * P : (do + 1) * P],
                            rhs=qT_sb[:, dki, :],
                            start=(dki == 0),
                            stop=(dki == n_do - 1),
                        )
                    nc.vector.tensor_scalar_add(
                        qhT_sb[:, do, :], pp, bq_sb[:, do : do + 1]
                    )
                return run

            chunks.append(tr_chunk)
            for do in range(n_do):
                chunks.append(proj_chunk(do))
            return qhT_sb, chunks

        def sc_exp(qhT_sb, g, ib):
            """Scores+exp for head pair g. Returns (es, poA, poB, pv, norm)."""
            h0, h1 = 2 * g, 2 * g + 1
            kh0 = khT_sb[0:DK, g, :]
            kh1 = khT_sb[DK:P, g, :]
            qh0 = qhT_sb[0:DK, g, :]
            qh1 = qhT_sb[DK:P, g, :]
            poA = psOA.tile([P, IB], f32, tag="poA", name=f"poA_{ib}_{g}")
            poB = psOB.tile([P, IB], f32, tag="poB", name=f"poB_{ib}_{g}")
            es = []

            def sc(sg):
                pa = psA.tile([P, 2, IB], f32, tag="psA", name=f"pa_{ib}_{g}_{sg}")
                pb = psB.tile([P, 2, IB], f32, tag="psB", name=f"pb_{ib}_{g}_{sg}")
                for u in range(2):
                    jt = sg * 2 + u
                    nc.tensor.matmul(
                        pa[:, u, :], lhsT=kh0[:, jt * P : (jt + 1) * P], rhs=qh0,
                        start=True, stop=True,
                    )
                    nc.tensor.matmul(
                        pb[:, u, :], lhsT=kh1[:, jt * P : (jt + 1) * P], rhs=qh1,
                        start=True, stop=True,
                    )
                e0 = e_pool.tile([P, 2, IB], bf16, tag="e0", name=f"e0_{ib}_{g}_{sg}")
                e1 = e_pool.tile([P, 2, IB], bf16, tag="e1", name=f"e1_{ib}_{g}_{sg}")
                nc.scalar.activation(e0, pa, Exp, scale=0.125)
                nc.scalar.activation(e1, pb, Exp, scale=0.125)
                es.append((e0, e1))

            def pv(sg):
                e0, e1 = es[sg]
                for u in range(2):
                    jt = sg * 2 + u
                    nc.tensor.matmul(
                        poA[:HW, :],
                        lhsT=vh_sb[:, jt, h0 * HW : (h0 + 1) * HW],
                        rhs=e0[:, u, :],
                        start=(jt == 0), stop=(jt == n_jt - 1),
                    )
                    nc.tensor.matmul(
                        poB[:HW, :],
                        lhsT=vh_sb[:, jt, h1 * HW : (h1 + 1) * HW],
                        rhs=e1[:, u, :],
                        start=(jt == 0), stop=(jt == n_jt - 1),
                    )

            def norm(outT_sb):
                for half, po, dst in (
                    ("A", poA, outT_sb[0:DK, g, :]),
                    ("B", poB, outT_sb[DK:P, g, :]),
                ):
                    rec = sm_pool.tile([1, IB], f32, tag=f"rec{half}", name=f"rc{half}_{ib}_{g}")
                    nc.vector.reciprocal(rec, po[DK : DK + 1, :])
                    srep = sm_pool.tile([DK, IB], f32, tag=f"srep{half}", name=f"sr{half}_{ib}_{g}")
                    nc.gpsimd.partition_broadcast(srep, rec)
                    nc.vector.tensor_tensor(dst, po[0:DK, :], srep, Alu.mult)

            return sc, pv, norm

        def head_pair(qhT_sb, outT_sb, g, ib):
            sc, pv, norm = sc_exp(qhT_sb, g, ib)
            for sg in range(n_jt // 2):
                sc(sg)
                if sg > 0:
                    pv(sg - 1)
            pv(n_jt // 2 - 1)
            norm(outT_sb)

        def make_final(outT_sb, ib):
            i0 = ib * IB
            chunks = []

            def fin_chunk(it, db):
                def run():
                    pf = ps_g.tile([P, 512], f32, tag="psg", name=f"pf_{ib}_{it}_{db}")
                    for dmo in range(n_do):
                        nc.tensor.matmul(
                            pf,
                            lhsT=outT_sb[:, dmo, it * P : (it + 1) * P],
                            rhs=Wo_sb[:, dmo, db * 512 : (db + 1) * 512],
                            start=(dmo == 0),
                            stop=(dmo == n_do - 1),
                        )
                    osb = o_pool.tile([P, 512], f32, tag="osb", name=f"ob_{ib}_{it}_{db}")
                    nc.vector.tensor_tensor(
                        osb, pf, bo_rep[:, db * 512 : (db + 1) * 512], Alu.add
                    )
                    nc.sync.dma_start(
                        out[i0 + it * P : i0 + (it + 1) * P, db * 512 : (db + 1) * 512],
                        osb,
                    )
                return run

            for it in range(n_it):
                for db in range(DM // 512):
                    chunks.append(fin_chunk(it, db))
            return chunks

        # ---------- emission ----------
        nc.sync.dma_start(bk_sb, bk.rearrange("(o p) -> p o", p=P))
        nc.sync.dma_start(bq_sb, bq.rearrange("(o p) -> p o", p=P))
        nc.sync.dma_start(Wk_sb, Wk.rearrange("(o p) f -> p o f", p=P))
        nc.sync.dma_start_transpose(kT_sb, k[:, :])
        nc.sync.dma_start(Wq_sb, Wq.rearrange("(o p) f -> p o f", p=P))

        e_pool = ctx.enter_context(tc.tile_pool(name="e", bufs=6))
        outT_pool = ctx.enter_context(tc.tile_pool(name="outT", bufs=2))
        sm_pool = ctx.enter_context(tc.tile_pool(name="sm", bufs=2))
        o_pool = ctx.enter_context(tc.tile_pool(name="osb", bufs=4))

        qhT_cur, prep0 = make_prep(0)
        prep0[0]()  # qT block-0 DMA-transpose
        nc.sync.dma_start(Wv_sb, Wv.rearrange("(o p) f -> p o f", p=P))
        nc.sync.dma_start_transpose(vT_sb, v[:, :])
        nc.sync.dma_start(bv_rep, bv[None, :].to_broadcast([P, DM]))
        nc.sync.dma_start(bo_rep, bo[None, :].to_broadcast([P, DM]))

        # pairs 0-1 of block 0: scores+exp now, PV deferred until after V-proj
        outT0 = outT_pool.tile([P, n_do, IB], bf16, tag="outT", name="oT_0")
        for do in range(4):
            kp_chunk(do)()
        prep0[1]()  # Q proj do=0
        sc0, pv0, norm0 = sc_exp(qhT_cur, 0, 0)
        for sg in range(n_jt // 2):
            sc0(sg)
        for do in range(4, n_do):
            kp_chunk(do)()
        prep0[2]()  # Q proj do=1
        sc1, pv1, norm1 = sc_exp(qhT_cur, 1, 0)
        sc1(0)
        sc1(1)  # e bufs=6: two groups of pair 1 can run ahead of pair-0 PV
        ones_view = vh_sb.rearrange("p o (h c) -> p o h c", c=HW)[:, :, :, DK]
        nc.vector.memset(ones_view, 1.0)
        for jt in range(n_jt):
            for db in range(DM // 512):
                vproj_chunk(jt, db)()
        nc.sync.dma_start(Wo_sb, Wo.rearrange("(o p) f -> p o f", p=P))
        for sg in range(n_jt // 2):
            pv0(sg)
        norm0(outT0)
        sc1(2)
        pv1(0)
        sc1(3)
        pv1(1)
        pv1(2)
        pv1(3)
        norm1(outT0)

        pending = []
        qhT_next = None
        for ib in range(n_ib):
            if ib == 0:
                outT_sb = outT0
            else:
                outT_sb = outT_pool.tile([P, n_do, IB], bf16, tag="outT", name=f"oT_{ib}")
            if ib + 1 < n_ib:
                qhT_next, prep_next = make_prep(ib + 1)
            else:
                qhT_next, prep_next = None, []
            extra = list(prep_next) + pending
            gs = list(range(2, HEADS // 2)) if ib == 0 else list(range(HEADS // 2))
            per = (len(extra) + len(gs) - 1) // len(gs)
            ei = 0
            for g in gs:
                if ib == 0:
                    prep0[1 + g]()  # this pair's Q-proj (dep of its scores)
                head_pair(qhT_cur, outT_sb, g, ib)
                for _ in range(per):
                    if ei < len(extra):
                        extra[ei]()
                        ei += 1
            while ei < len(extra):
                extra[ei]()
                ei += 1
            pending = make_final(outT_sb, ib)
            qhT_cur = qhT_next
        for c in pending:
            c()

    nc.compile()
    return nc


def _get_nc(lq=LQ, lkv=LKV):
    key = (lq, lkv)
    if key not in _cache:
        _cache[key] = _build_nc(lq, lkv)
    return _cache[key]


def kernel(**inputs):
    import ml_dtypes

    from concourse.bass_utils import run_bass_kernel_spmd

    bf16 = ml_dtypes.bfloat16
    nc = _get_nc()
    shared = {}
    for n in ("Wq", "Wk", "Wv", "Wo"):
        shared[n] = np.ascontiguousarray(np.asarray(inputs[n]).astype(bf16))
    for n in ("bq", "bk", "bv", "bo"):
        shared[n] = np.ascontiguousarray(np.asarray(inputs[n], dtype=np.float32))
    in_maps = []
    for b in range(B):
        m = dict(shared)
        for n in ("q", "k", "v"):
            m[n] = np.ascontiguousarray(np.asarray(inputs[n])[b].astype(bf16))
        in_maps.append(m)
    res = run_bass_kernel_spmd(nc, in_maps, list(range(B)), trace=TRACE)
    global LAST_RESULT
    LAST_RESULT = res
    return np.stack([res.results[b]["out"] for b in range(B)], axis=0)
